# revision 16
# baseline (speedup 1.0000x reference)
"""DeepFM forward kernel for 8 Trainium2 NeuronCores (Bass/Tile).

Single-phase data-parallel design (batch split 8 ways, 2048 rows/core):
  - Fields 0/1 (vocab 31360/6807): 32 [P,1]-index SWDGE gathers from a
    combined bf16 table [emb(128) | fc | pad] (132-wide rows), using
    host-precomputed global ids. fc rides along with the embedding row.
  - Fields 2/3 (vocab 18/94): no gather - one-hot matmuls on the PE
    produce the feature-major embeddings directly, and a combined-vocab
    one-hot against a host-precomputed per-row [fc, rowsum, rowsumsq]
    table yields their linear-term and FM-stat contributions.
  - Gathered rows are DMA-transposed (XBAR, all on the sync queue,
    which carries nothing else) to feature-major and cast to fp8e4;
    the 3-layer MLP runs in fp8 DoubleRow mode (weights pre-scaled x64
    host-side, undone in the activation scale), f32 PSUM.
  - The FM quirk term 0.5*sum_B(rowsum^2 - rowssq) is a global scalar:
    per-core partials are all-reduced ON-DEVICE while the MLP runs. The
    whole reduce chain (partition reduction via DRAM round-trip, bounce
    buffers, collective, readback) lives on the gpsimd queue so no
    compute engine ever blocks on it.
  - Tail: per-chunk L4 matmuls bank pre-sigmoid logits in SBUF; one
    final sigmoid pass applies lin + 0.5*g + bias + b4.
"""

import os
import numpy as np

# ---- problem constants (hardcoded; kernel.py must be self-contained) ----
CAT_SIZES = [31360, 6807, 18, 94]
TOTAL = 38279
S0, S1, S2, S3 = CAT_SIZES
N01 = S0 + S1                  # rows in the fields-0/1 combined table
EMB = 128
F = 4
B = 16384
N_CORES = 8
P = 128
CW = 132                       # combined table row width (emb 128 | fc | pad)
SC = 64.0                      # fp8 weight pre-scale
OFFSETS_NP = np.array([0, 31360, 38167, 38185], dtype=np.int32)

_build_cache = {}


def _build(b_loc, n_cores):
    import concourse.bass as bass
    import concourse.mybir as mybir
    import concourse.tile as tile
    from concourse import bacc

    f32 = mybir.dt.float32
    bf16 = mybir.dt.bfloat16
    fp8 = mybir.dt.float8e4
    i32 = mybir.dt.int32
    AF = mybir.ActivationFunctionType
    ALU = mybir.AluOpType
    AX = mybir.AxisListType
    DR = mybir.MatmulPerfMode.DoubleRow

    NCH = b_loc // P               # 16 chunks of 128 rows
    NB = 512                       # matmul moving width
    NJ = b_loc // NB               # 4 j-blocks
    CPJ = NB // P                  # 4 chunks per j-block

    nc = bacc.Bacc(
        "TRN2",
        target_bir_lowering=False,
        debug=False,
        num_devices=n_cores,
    )

    # ---- DRAM I/O ----
    ctab_d = nc.dram_tensor("ctab01", [N01, CW], bf16, kind="ExternalInput").ap()
    xg_d = nc.dram_tensor("xg", [b_loc, 2], i32, kind="ExternalInput").ap()
    tab2_d = nc.dram_tensor("tab2", [P, P], bf16, kind="ExternalInput").ap()
    tab3_d = nc.dram_tensor("tab3s", [P, P], bf16, kind="ExternalInput").ap()
    stc_d = nc.dram_tensor("stc", [P, 3], bf16, kind="ExternalInput").ap()
    bc2_d = nc.dram_tensor("bc2", [P, b_loc], bf16, kind="ExternalInput").ap()
    bc3_d = nc.dram_tensor("bc3s", [P, b_loc], bf16, kind="ExternalInput").ap()
    iota_d = nc.dram_tensor("iota", [P, 1], bf16, kind="ExternalInput").ap()
    W1_d = nc.dram_tensor("W1s", [P, 4, 2048], fp8, kind="ExternalInput").ap()
    W2_d = nc.dram_tensor("W2s", [P, 16, 1024], fp8, kind="ExternalInput").ap()
    W3_d = nc.dram_tensor("W3s", [P, 8, 512], fp8, kind="ExternalInput").ap()
    W4_d = nc.dram_tensor("W4s", [P, 4, 1], fp8, kind="ExternalInput").ap()
    b1_d = nc.dram_tensor("b1p", [P, 16], f32, kind="ExternalInput").ap()
    b2_d = nc.dram_tensor("b2p", [P, 8], f32, kind="ExternalInput").ap()
    b3_d = nc.dram_tensor("b3p", [P, 4], f32, kind="ExternalInput").ap()
    bb4_d = nc.dram_tensor("bias_b4", [1, 1], f32, kind="ExternalInput").ap()
    y_d = nc.dram_tensor("y", [b_loc, 1], f32, kind="ExternalOutput").ap()

    MT1, MT2, MT3 = 16, 8, 4       # m-tiles per layer
    KP1, KP2, KP3 = 2, 8, 4        # DoubleRow k-pairs per layer

    with tile.TileContext(nc) as tc:
        with (
            tc.tile_pool(name="const", bufs=1) as const,
            tc.tile_pool(name="work", bufs=4) as work,
            tc.tile_pool(name="acts", bufs=2) as acts,
            tc.tile_pool(name="psmm", bufs=6, space="PSUM") as psmm,
            tc.tile_pool(name="pssm", bufs=2, space="PSUM") as pssm,
            tc.tile_pool(name="dram", bufs=4, space="DRAM") as dram,
        ):
            # ---- sync queue: xi + L1-critical weights, then transposes ----
            xi = const.tile([P, NCH, 2], i32, tag="xi")
            nc.sync.dma_start(xi[:], xg_d.rearrange("(c p) f -> p c f", p=P))
            W1s = const.tile([P, 4, 2048], fp8, tag="W1s")
            nc.sync.dma_start(W1s[:], W1_d)
            b1p = const.tile([P, MT1], f32, tag="b1p")
            nc.sync.dma_start(b1p[:], b1_d)
            W2s = const.tile([P, 16, 1024], fp8, tag="W2s")
            nc.sync.dma_start(W2s[:], W2_d)

            # ---- scalar queue: small consts, then only activations ----
            bc2_sb = const.tile([P, b_loc], bf16, tag="bc2")
            nc.scalar.dma_start(bc2_sb[:], bc2_d)
            bc3_sb = const.tile([P, b_loc], bf16, tag="bc3")
            nc.scalar.dma_start(bc3_sb[:], bc3_d)
            iota_sb = const.tile([P, 1], bf16, tag="iota")
            nc.scalar.dma_start(iota_sb[:], iota_d)
            tab2_sb = const.tile([P, P], bf16, tag="tab2")
            nc.scalar.dma_start(tab2_sb[:], tab2_d)
            tab3_sb = const.tile([P, P], bf16, tag="tab3")
            nc.scalar.dma_start(tab3_sb[:], tab3_d)
            stc_sb = const.tile([P, 3], bf16, tag="stc")
            nc.scalar.dma_start(stc_sb[:], stc_d)
            W3s = const.tile([P, 8, 512], fp8, tag="W3s")
            nc.scalar.dma_start(W3s[:], W3_d)
            W4s = const.tile([P, 4, 1], fp8, tag="W4s")
            nc.scalar.dma_start(W4s[:], W4_d)
            b2p = const.tile([P, MT2], f32, tag="b2p")
            nc.scalar.dma_start(b2p[:], b2_d)
            b3p = const.tile([P, MT3], f32, tag="b3p")
            nc.scalar.dma_start(b3p[:], b3_d)
            bb4_sb = const.tile([1, 1], f32, tag="bb4")
            nc.scalar.dma_start(bb4_sb[:], bb4_d)
            ones_row_f = const.tile([1, P], f32, tag="ones_row_f")
            nc.vector.memset(ones_row_f[:], 1.0)

            # ---- gathers: fields 0/1, one [P,1] call per (chunk, field) ----
            G01 = const.tile([P, NCH, 2, CW], bf16, tag="G01")
            for c in range(NCH):
                for f in range(2):
                    nc.gpsimd.indirect_dma_start(
                        out=G01[:, c, f, :],
                        out_offset=None,
                        in_=ctab_d,
                        in_offset=bass.IndirectOffsetOnAxis(
                            ap=xi[:, c, f:f + 1], axis=0
                        ),
                    )

            # ---- one-hots for fields 2/3 (vector) ----
            embT8 = const.tile([P, F, b_loc], fp8, tag="embT8")
            oh2 = const.tile([P, b_loc], bf16, tag="oh2")
            nc.vector.tensor_tensor(
                out=oh2[:], in0=bc2_sb[:],
                in1=iota_sb[:].to_broadcast([P, b_loc]), op=ALU.is_equal,
            )
            oh3 = const.tile([P, b_loc], bf16, tag="oh3")
            nc.vector.tensor_tensor(
                out=oh3[:], in0=bc3_sb[:],
                in1=iota_sb[:].to_broadcast([P, b_loc]), op=ALU.is_equal,
            )
            ohc = const.tile([P, b_loc], bf16, tag="ohc")
            nc.vector.tensor_tensor(
                out=ohc[:], in0=oh2[:], in1=oh3[:], op=ALU.add,
            )

            # fields 2/3 [fc, rowsum, rowsumsq] per chunk (combined vocab)
            st23 = const.tile([P, NCH, 3], f32, tag="st23")
            for c in range(NCH):
                csl = slice(c * P, (c + 1) * P)
                ps3 = pssm.tile([P, 4], f32, tag="sm", name=f"st_{c}")
                nc.tensor.matmul(
                    ps3[:, 0:3], lhsT=ohc[:, csl], rhs=stc_sb[:],
                    start=True, stop=True,
                )
                nc.vector.tensor_copy(st23[:, c, :], ps3[:, 0:3])

            # fields 2/3 embeddings (feature-major direct)
            for fi, tab, o in ((2, tab2_sb, oh2), (3, tab3_sb, oh3)):
                for j in range(NJ):
                    jsl = slice(j * NB, (j + 1) * NB)
                    pse = psmm.tile([P, NB], f32, tag="mm")
                    nc.tensor.matmul(
                        pse[:], lhsT=tab[:], rhs=o[:, jsl],
                        start=True, stop=True,
                    )
                    nc.vector.tensor_copy(embT8[:, fi, jsl], pse[:])

            # ---- fields 0/1: transpose + cast first, then FM stats ----
            rs01 = const.tile([P, NCH, 2], f32, tag="rs01")
            rq01 = const.tile([P, NCH, 2], f32, tag="rq01")
            for c in range(NCH):
                for f in range(2):
                    tb = work.tile([P, P], bf16, tag="tb", name=f"tb_{c}_{f}")
                    nc.sync.dma_start_transpose(tb[:], G01[:, c, f, 0:EMB])
                    nc.vector.tensor_copy(
                        embT8[:, f, c * P:(c + 1) * P], tb[:]
                    )
                for f in range(2):
                    nc.vector.reduce_sum(
                        out=rs01[:, c, f:f + 1],
                        in_=G01[:, c, f, 0:EMB], axis=AX.X,
                    )
                    sq = work.tile([P, EMB], f32, tag="sq", name=f"sq_{c}_{f}")
                    nc.vector.tensor_tensor(
                        out=sq[:], in0=G01[:, c, f, 0:EMB],
                        in1=G01[:, c, f, 0:EMB], op=ALU.mult,
                    )
                    nc.vector.reduce_sum(
                        out=rq01[:, c, f:f + 1], in_=sq[:], axis=AX.X,
                    )

            # ---- FM combine (vector) -> collective chain (gpsimd) ----
            lin = const.tile([P, NCH], f32, tag="lin")
            nc.vector.tensor_tensor(
                out=lin[:], in0=G01[:, :, 0, EMB], in1=G01[:, :, 1, EMB],
                op=ALU.add,
            )
            nc.vector.tensor_tensor(
                out=lin[:], in0=lin[:], in1=st23[:, :, 0], op=ALU.add,
            )
            rs = const.tile([P, NCH], f32, tag="rs")
            nc.vector.tensor_tensor(
                out=rs[:], in0=rs01[:, :, 0], in1=rs01[:, :, 1], op=ALU.add,
            )
            nc.vector.tensor_tensor(
                out=rs[:], in0=rs[:], in1=st23[:, :, 1], op=ALU.add,
            )
            rq = const.tile([P, NCH], f32, tag="rq")
            nc.vector.tensor_tensor(
                out=rq[:], in0=rq01[:, :, 0], in1=rq01[:, :, 1], op=ALU.add,
            )
            nc.vector.tensor_tensor(
                out=rq[:], in0=rq[:], in1=st23[:, :, 2], op=ALU.add,
            )
            sosd = const.tile([P, NCH], f32, tag="sosd")
            nc.vector.tensor_tensor(
                out=sosd[:], in0=rs[:], in1=rs[:], op=ALU.mult,
            )
            nc.vector.tensor_tensor(
                out=sosd[:], in0=sosd[:], in1=rq[:], op=ALU.subtract,
            )
            pg = const.tile([P, 1], f32, tag="pg")
            nc.vector.reduce_sum(out=pg[:], in_=sosd[:], axis=AX.X)
            # partition reduce on gpsimd (off every critical queue)
            g_sb = const.tile([1, 1], f32, tag="g_sb")
            nc.gpsimd.reduce_sum(out=g_sb[:], in_=pg[:], axis=AX.C)
            in_b = dram.tile([1, 1], f32)
            out_b = dram.tile([1, 1], f32)
            nc.gpsimd.dma_start(in_b[:], g_sb[:])
            nc.gpsimd.collective_compute(
                "AllReduce",
                mybir.AluOpType.add,
                replica_groups=[list(range(n_cores))],
                ins=[in_b.opt()],
                outs=[out_b.opt()],
            )
            g_all = const.tile([1, 1], f32, tag="g_all")
            nc.gpsimd.dma_start(g_all[:], out_b[:])
            # S = 0.5*g + bias + b4 (computed on gpsimd to keep scalar free)
            S1 = const.tile([1, 1], f32, tag="S1")
            nc.gpsimd.tensor_scalar(
                out=S1[:], in0=g_all[:], scalar1=0.5, scalar2=None,
                op0=ALU.mult,
            )
            nc.gpsimd.tensor_tensor(
                out=S1[:], in0=S1[:], in1=bb4_sb[:], op=ALU.add,
            )

            # ---- fp8 DoubleRow MLP; pre-sigmoid logits banked in zsb ----
            zsb = const.tile([P, NCH], f32, tag="zsb")
            Sbc = const.tile([P, 1], f32, tag="Sbc")
            ISC = float(1.0 / SC)
            layers = [
                (KP1, MT1, W1s, b1p, "h1"),
                (KP2, MT2, W2s, b2p, "h2"),
                (KP3, MT3, W3s, b3p, "h3"),
            ]
            for j in range(NJ):
                jsl = slice(j * NB, (j + 1) * NB)
                h_prev = embT8[:, :, jsl]
                for (KP, MT, Ws, bp, lname) in layers:
                    h_next = acts.tile([P, MT, NB], fp8, tag=lname,
                                       name=f"{lname}_{j}")
                    for m in range(MT):
                        ps = psmm.tile([P, NB], f32, tag="mm")
                        for t in range(KP):
                            nc.tensor.matmul(
                                ps[:],
                                lhsT=Ws[:, 2 * t:2 * t + 2,
                                        m * P:(m + 1) * P],
                                rhs=h_prev[:, 2 * t:2 * t + 2, :],
                                start=(t == 0),
                                stop=(t == KP - 1),
                                perf_mode=DR,
                            )
                        nc.scalar.activation(
                            h_next[:, m, :], ps[:], AF.Relu,
                            bias=bp[:, m:m + 1], scale=ISC,
                        )
                    h_prev = h_next[:]
                # L4 (K=512, N=1) per chunk; bank logits for the final pass
                for cs in range(CPJ):
                    c = j * CPJ + cs
                    ps4 = pssm.tile([P, 4], f32, tag="sm", name=f"l4_{c}")
                    for k in range(4):
                        nc.tensor.matmul(
                            ps4[:, 0:1],
                            lhsT=h_prev[:, k, cs * P:(cs + 1) * P],
                            rhs=W4s[:, k, :],
                            start=(k == 0),
                            stop=(k == 3),
                        )
                    nc.vector.tensor_copy(zsb[:, c:c + 1], ps4[:, 0:1])

            # S broadcast to partitions (PE reaches this after j3, by which
            # time the collective has long finished)
            Sps = pssm.tile([P, 4], f32, tag="sm", name="Sps")
            nc.tensor.matmul(
                Sps[:, 0:1], lhsT=ones_row_f[:], rhs=S1[:],
                start=True, stop=True,
            )
            nc.vector.tensor_copy(Sbc[:], Sps[:, 0:1])

            # ---- final tail: sigmoid((zsb + SC*(lin + S)) / SC) ----
            linS = const.tile([P, NCH], f32, tag="linS")
            nc.vector.tensor_tensor(
                out=linS[:], in0=lin[:], in1=Sbc[:].to_broadcast([P, NCH]),
                op=ALU.add,
            )
            nc.vector.tensor_scalar(
                out=linS[:], in0=linS[:], scalar1=SC, scalar2=None,
                op0=ALU.mult,
            )
            nc.vector.tensor_tensor(
                out=zsb[:], in0=zsb[:], in1=linS[:], op=ALU.add,
            )
            ysb = const.tile([P, NCH], f32, tag="ysb")
            nc.scalar.activation(ysb[:], zsb[:], AF.Sigmoid, scale=ISC)

            nc.sync.dma_start(y_d.rearrange("(c p) o -> p (c o)", p=P),
                              ysb[:])

    nc.compile()
    return nc


def _get_program(b_loc, n_cores):
    key = (b_loc, n_cores)
    if key not in _build_cache:
        _build_cache[key] = _build(b_loc, n_cores)
    return _build_cache[key]


def _prep_shared(inputs):
    """Host-side table/weight prep (replicated across cores)."""
    import ml_dtypes
    bf = ml_dtypes.bfloat16
    f8 = ml_dtypes.float8_e4m3

    emb = np.asarray(inputs["emb_table"], np.float32)
    fc = np.asarray(inputs["fc"], np.float32).reshape(-1)

    ctab = np.zeros((N01, CW), np.float32)
    ctab[0:S0, 0:EMB] = emb[0:S0]
    ctab[0:S0, EMB] = fc[0:S0]
    ctab[S0:N01, 0:EMB] = emb[0:S1]
    ctab[S0:N01, EMB] = fc[S0:N01]

    tab2 = np.zeros((P, P), np.float32)
    tab2[0:S2] = emb[0:S2]
    tab3s = np.zeros((P, P), np.float32)
    tab3s[S2:S2 + S3] = emb[0:S3]          # field-3 rows shifted by S2
    stc = np.zeros((P, 3), np.float32)
    stc[0:S2, 0] = fc[OFFSETS_NP[2]:OFFSETS_NP[2] + S2]
    stc[0:S2, 1] = emb[0:S2].sum(axis=1)
    stc[0:S2, 2] = (emb[0:S2] ** 2).sum(axis=1)
    stc[S2:S2 + S3, 0] = fc[OFFSETS_NP[3]:OFFSETS_NP[3] + S3]
    stc[S2:S2 + S3, 1] = emb[0:S3].sum(axis=1)
    stc[S2:S2 + S3, 2] = (emb[0:S3] ** 2).sum(axis=1)

    def wtile(W, ksub):
        W = np.asarray(W, np.float32)
        k, m = W.shape
        t = W.reshape(ksub, P, m).transpose(1, 0, 2) * SC
        return np.ascontiguousarray(t).astype(f8)

    shared = {
        "ctab01": np.ascontiguousarray(ctab).astype(bf),
        "tab2": tab2.astype(bf),
        "tab3s": tab3s.astype(bf),
        "stc": stc.astype(bf),
        "iota": np.arange(P, dtype=np.float32).reshape(P, 1).astype(bf),
        "W1s": wtile(inputs["W1"], 4),
        "W2s": wtile(inputs["W2"], 16),
        "W3s": wtile(inputs["W3"], 8),
        "W4s": wtile(inputs["W4"], 4),
        "b1p": np.ascontiguousarray(
            np.asarray(inputs["b1"], np.float32).reshape(16, P).T),
        "b2p": np.ascontiguousarray(
            np.asarray(inputs["b2"], np.float32).reshape(8, P).T),
        "b3p": np.ascontiguousarray(
            np.asarray(inputs["b3"], np.float32).reshape(4, P).T),
        "bias_b4": np.asarray(
            np.asarray(inputs["bias"], np.float32).reshape(-1)[0]
            + np.asarray(inputs["b4"], np.float32).reshape(-1)[0]
        ).reshape(1, 1).astype(np.float32),
    }
    return shared


def make_in_maps(inputs, b_loc, n_cores):
    import ml_dtypes
    bf = ml_dtypes.bfloat16

    shared = _prep_shared(inputs)
    x_int = np.asarray(inputs["x"], np.float32).astype(np.int32)  # [B, F]

    in_maps = []
    for c in range(n_cores):
        xs = x_int[c * b_loc:(c + 1) * b_loc]
        xg = np.stack([xs[:, 0], xs[:, 1] + S0], axis=1).astype(np.int32)
        m = dict(shared)
        m["xg"] = np.ascontiguousarray(xg)
        m["bc2"] = np.ascontiguousarray(np.broadcast_to(
            xs[:, 2].astype(np.float32)[None, :], (P, b_loc))).astype(bf)
        m["bc3s"] = np.ascontiguousarray(np.broadcast_to(
            (xs[:, 3] + S2).astype(np.float32)[None, :],
            (P, b_loc))).astype(bf)
        in_maps.append(m)
    return in_maps


def kernel(**inputs) -> np.ndarray:
    from concourse.bass_utils import run_bass_kernel_spmd

    n_cores = N_CORES
    b_loc = B // n_cores
    cores = list(range(n_cores))
    trace = bool(int(os.environ.get("KERNEL_TRACE", "0")))

    nc = _get_program(b_loc, n_cores)
    res = run_bass_kernel_spmd(
        nc, make_in_maps(inputs, b_loc, n_cores), core_ids=cores, trace=trace,
    )
    kernel._last_results = res
    kernel._last_exec_ns = res.exec_time_ns
    out = np.concatenate([np.asarray(r["y"]) for r in res.results], axis=0)
    return out.astype(np.float32)
